# revision 14
# baseline (speedup 1.0000x reference)
"""Trainium2 Bass kernel for nn_DeepseekV4Compressor (scatter_memory).

Computation (per the problem's nn.Module):
  w        = weight * block_scale_expand(weight_scale)     # fp8-style dequant
  kv_score = hidden_states @ w.T                           # [8192, 1024]
  kv, score = kv_score[:, :512], kv_score[:, 512:]
  state    = [kv + ape[pos%4] | score]  scattered into the paged state cache
  per compression window of R=4 tokens:
    gate = softmax(score_window, axis=window)  (per channel)
    comp = sum(gate * kvp)                                  # [2048, 512]
    x    = RoPE(RMSNorm(comp per 256-ch head), cos_sin[pos//4])
    kc[kv_slot_mapping[window_end]] = x

Sharding: tokens split contiguously across 8 cores (1024 tokens = half of one
request per core, so compression windows never straddle cores and all
scatter sources stay local).  The matmul runs in float32r (fp32 operands
truncated to ~fp22 in the PE) at full PE rate.  hidden is transposed on-chip
with PE transpose-mode matmuls; the weight is passed pre-transposed (pure
host-side relayout) and block-dequanted on the vector engine.  The window
softmax reduction (sum over 4 consecutive tokens living on adjacent
partitions) is computed as PE matmuls against a constant block-ones matrix.
The paged-cache scatters are applied during unsharding with the integer index
tensors (each row is produced exactly once, so scatter order is irrelevant).
"""

import numpy as np

NCORES = 8
T, H = 8192, 4096
D = 512            # coff * head_dim
D2 = 2 * D         # kv | score
HD = 256           # head dim
R = 4              # compression ratio
TPC = T // NCORES  # tokens per core = 1024
WPC = TPC // R     # windows per core = 256
NKB = H // 128     # 32 k-tiles of 128
NTT = TPC // 128   # 8 token tiles per core
NWT = WPC // 128   # 2 window tiles per core
NG = NKB // 4      # 8 transpose groups (4 k-tiles each) per token tile
EPS = 1e-6

_cache = {}


def _build_program():
    import concourse.bass as bass
    import concourse.tile as tile
    from concourse import bacc, mybir

    f32 = mybir.dt.float32
    f32r = mybir.dt.float32r
    AF = mybir.ActivationFunctionType

    nc = bacc.Bacc(
        "TRN2", target_bir_lowering=False, debug=False, num_devices=NCORES
    )

    h = nc.dram_tensor("h", [TPC, H], f32, kind="ExternalInput").ap()
    wt = nc.dram_tensor("wt", [H, D2], f32, kind="ExternalInput").ap()
    wsc = nc.dram_tensor("wsc", [128, NKB * 8], f32, kind="ExternalInput").ap()
    ape_r = nc.dram_tensor("ape_r", [128, D], f32, kind="ExternalInput").ap()
    cs2 = nc.dram_tensor("cs2", [WPC, 4 * 128], f32, kind="ExternalInput").ap()
    amat = nc.dram_tensor("amat", [128, 4 * 128], f32, kind="ExternalInput").ap()
    ident = nc.dram_tensor("ident", [128, 128], f32, kind="ExternalInput").ap()

    kv_score = nc.dram_tensor("kv_score", [TPC, D2], f32, kind="ExternalOutput").ap()
    state = nc.dram_tensor("state", [TPC, D2], f32, kind="ExternalOutput").ap()
    flat = nc.dram_tensor("flat", [WPC, D], f32, kind="ExternalOutput").ap()

    with tile.TileContext(nc) as tc:
        with (
            tc.tile_pool(name="consts", bufs=1) as consts,
            tc.tile_pool(name="wpool", bufs=1) as wpool,
            tc.tile_pool(name="hn", bufs=6) as hpool,
            tc.tile_pool(name="wst", bufs=2) as wstp,
            tc.tile_pool(name="ht", bufs=3) as htp,
            tc.tile_pool(name="ep", bufs=3) as epool,
            tc.tile_pool(name="pp", bufs=3) as ppool,
            tc.tile_pool(name="sp", bufs=2) as spool,
            tc.tile_pool(name="wep", bufs=1) as wepool,
            tc.tile_pool(name="tmp", bufs=1) as tmpp,
            tc.tile_pool(name="tp", bufs=2, space="PSUM") as tpp,
            tc.tile_pool(name="acc", bufs=4, space="PSUM") as accp,
            tc.tile_pool(name="win", bufs=2, space="PSUM") as winp,
        ):
            # ---- constants ----
            wsc_sb = consts.tile([128, NKB * 8], f32)
            nc.sync.dma_start(wsc_sb[:], wsc[:])
            ape_sb = consts.tile([128, D], f32)
            nc.sync.dma_start(ape_sb[:], ape_r[:])
            cs_sb = consts.tile([128, NWT, 4 * 128], f32)
            nc.sync.dma_start(cs_sb[:], cs2.rearrange("(a p) d -> p a d", p=128))
            am_r = consts.tile([128, 4 * 128], f32r)
            am_st = wstp.tile([128, 4 * 128], f32, tag="wst", name="am_st")
            nc.sync.dma_start(am_st[:], amat[:])
            nc.vector.tensor_copy(am_r[:], am_st[:])
            id_sb = consts.tile([128, 128], f32)
            nc.sync.dma_start(id_sb[:], ident[:])
            eps_sb = consts.tile([128, 1], f32)
            nc.vector.memset(eps_sb[:], EPS)

            # ---- weight: load transposed layout, dequant in place ----
            # wT_sb[p, kb*D2 + c] = w_deq[c, kb*128 + p]
            wT = wpool.tile([128, NKB * D2], f32r)
            for kb in range(NKB):
                wstage = wstp.tile([128, D2], f32, tag="wst", name=f"wst{kb}")
                nc.sync.dma_start(
                    wstage[:], wt[kb * 128 : (kb + 1) * 128, :]
                )
                seg_out = wT[:, kb * D2 : (kb + 1) * D2].rearrange(
                    "p (cb cc) -> p cb cc", cb=8
                )
                scl = (
                    wsc_sb[:, kb * 8 : (kb + 1) * 8]
                    .unsqueeze(2)
                    .broadcast_to([128, 8, 128])
                )
                nc.vector.tensor_mul(
                    seg_out,
                    wstage[:].rearrange("p (cb cc) -> p cb cc", cb=8),
                    scl,
                )

            def w_ap(kb, chunk):
                return wT[:, kb * D2 + chunk * D : kb * D2 + (chunk + 1) * D]

            # ---- main pipeline: K-outer over token-tile pairs ----
            # unit u = (pair p, k-group g); each unit transposes + matmuls
            # k-tiles g*4..g*4+3 for both tiles of the pair, so the weight
            # stream is consumed at half the per-tile rate (keeps the PE fed
            # during the initial weight DMA+dequant phase).
            NPAIR = NTT // 2
            hn_tiles = {}

            def load_hn(u):
                p, g = divmod(u, NG)
                for idx in range(2):
                    tt = 2 * p + idx
                    t = hpool.tile(
                        [128, 512], f32, tag="hn", name=f"hn{tt}_{g}"
                    )
                    eng = nc.sync if idx == 0 else nc.scalar
                    eng.dma_start(
                        t[:],
                        h[tt * 128 : (tt + 1) * 128, g * 512 : (g + 1) * 512],
                    )
                    hn_tiles[(tt, g)] = t

            ht_tiles = {}

            def emit_transpose_group(u, idx):
                p, g = divmod(u, NG)
                tt = 2 * p + idx
                hn = hn_tiles.pop((tt, g))
                pst = tpp.tile([128, 512], f32, tag="tp", name=f"pst{tt}_{g}")
                for j in range(4):
                    nc.tensor.matmul(
                        pst[:, j * 128 : (j + 1) * 128],
                        hn[:, j * 128 : (j + 1) * 128],
                        id_sb[:],
                        is_transpose=True,
                        skip_group_check=True,
                    )
                ht = htp.tile([128, 512], f32r, tag="ht", name=f"ht{tt}_{g}")
                nc.any.tensor_copy(out=ht[:], in_=pst[:])
                ht_tiles[(u, idx)] = ht

            ep_tiles = {}

            def emit_epilogue(tt, pkv, psc):
                kvsc = spool.tile([128, 3 * D], f32, tag="sp")
                nc.any.tensor_copy(out=kvsc[:, 0:D], in_=pkv[:])
                nc.any.tensor_copy(out=kvsc[:, D : 2 * D], in_=psc[:])
                nc.vector.tensor_add(kvsc[:, 2 * D : 3 * D], pkv[:], ape_sb[:])
                rows = slice(tt * 128, (tt + 1) * 128)
                nc.scalar.dma_start(kv_score[rows, :], kvsc[:, 0 : 2 * D])
                nc.scalar.dma_start(state[rows, 0:D], kvsc[:, 2 * D : 3 * D])
                nc.scalar.dma_start(state[rows, D : 2 * D], kvsc[:, D : 2 * D])
                E = epool.tile([128, D], f32r, tag="ep")
                nc.scalar.activation(E[:], psc[:], AF.Exp)
                P = ppool.tile([128, D], f32r, tag="pp")
                nc.vector.tensor_mul(
                    P[:], E[:].bitcast(f32), kvsc[:, 2 * D : 3 * D]
                )
                ep_tiles[tt] = (E, P)

            win_ps = {}

            def emit_win_mm(src_tt):
                wi, i = divmod(src_tt, 4)
                if i == 0:
                    win_ps[wi] = (
                        winp.tile([128, D], f32, tag="win", name=f"psN{wi}"),
                        winp.tile([128, D], f32, tag="win", name=f"psD{wi}"),
                    )
                psN, psD = win_ps[wi]
                E, P = ep_tiles.pop(src_tt)
                a_i = am_r[:, i * 128 : (i + 1) * 128]
                nc.tensor.matmul(psN, a_i, P[:], start=(i == 0), stop=(i == 3))
                nc.tensor.matmul(psD, a_i, E[:], start=(i == 0), stop=(i == 3))
                if i == 3:
                    emit_win_epilogue(wi)

            def emit_win_epilogue(wi):
                psN, psD = win_ps.pop(wi)
                rec = wepool.tile([128, D], f32, tag="rec")
                nc.vector.reciprocal(rec[:], psD[:])
                comp = wepool.tile([128, D], f32, tag="comp")
                nc.vector.tensor_mul(comp[:], psN[:], rec[:])
                sq = wepool.tile([128, HD], f32, tag="sq")
                ms = wepool.tile([128, 2], f32, tag="ms")
                nc.scalar.activation(
                    sq[:], comp[:, 0:HD], AF.Square, accum_out=ms[:, 0:1]
                )
                nc.scalar.activation(
                    sq[:], comp[:, HD : 2 * HD], AF.Square, accum_out=ms[:, 1:2]
                )
                rms = wepool.tile([128, 2], f32, tag="rms")
                nc.scalar.activation(rms[:], ms[:], AF.Sqrt, scale=1.0 / HD, bias=eps_sb[:])
                inv = wepool.tile([128, 2], f32, tag="inv")
                nc.vector.reciprocal(inv[:], rms[:])
                fl = wepool.tile([128, D], f32, tag="fl")
                for hh in range(2):
                    css = wepool.tile([128, 4 * 128], f32, tag="css")
                    nc.vector.tensor_scalar_mul(
                        css[:], cs_sb[:, wi], inv[:, hh : hh + 1]
                    )
                    x1 = comp[:, hh * HD : hh * HD + 128]
                    x2 = comp[:, hh * HD + 128 : (hh + 1) * HD]
                    t1 = tmpp.tile([128, 128], f32, tag="t1")
                    t2 = tmpp.tile([128, 128], f32, tag="t2")
                    nc.vector.tensor_mul(t1[:], x1, css[:, 0:128])
                    nc.vector.tensor_mul(t2[:], x2, css[:, 128:256])
                    nc.vector.tensor_sub(
                        fl[:, hh * HD : hh * HD + 128], t1[:], t2[:]
                    )
                    t3 = tmpp.tile([128, 128], f32, tag="t3")
                    t4 = tmpp.tile([128, 128], f32, tag="t4")
                    nc.vector.tensor_mul(t3[:], x2, css[:, 256:384])
                    nc.vector.tensor_mul(t4[:], x1, css[:, 384:512])
                    nc.vector.tensor_add(
                        fl[:, hh * HD + 128 : (hh + 1) * HD], t3[:], t4[:]
                    )
                nc.scalar.dma_start(flat[wi * 128 : (wi + 1) * 128, :], fl[:])

            def emit_mm(u, idx):
                p, g = divmod(u, NG)
                tt = 2 * p + idx
                pkv, psc = acc[tt]
                ht = ht_tiles.pop((u, idx))
                for j in range(4):
                    kb = g * 4 + j
                    lhs = ht[:, j * 128 : (j + 1) * 128]
                    nc.tensor.matmul(
                        pkv, lhs, w_ap(kb, 0), start=(kb == 0), stop=(kb == NKB - 1)
                    )
                    nc.tensor.matmul(
                        psc, lhs, w_ap(kb, 1), start=(kb == 0), stop=(kb == NKB - 1)
                    )

            total = NPAIR * NG
            acc = {}
            load_hn(0)
            load_hn(1)
            emit_transpose_group(0, 0)
            emit_transpose_group(0, 1)
            for u in range(total):
                p, g = divmod(u, NG)
                if g == 0:
                    for idx in range(2):
                        tt = 2 * p + idx
                        acc[tt] = (
                            accp.tile([128, D], f32, tag="acc", name=f"pkv{tt}"),
                            accp.tile([128, D], f32, tag="acc", name=f"psc{tt}"),
                        )
                if u + 2 < total:
                    load_hn(u + 2)
                if u + 1 < total:
                    emit_transpose_group(u + 1, 0)
                emit_mm(u, 0)
                if u + 1 < total:
                    emit_transpose_group(u + 1, 1)
                emit_mm(u, 1)
                if g == 1 and p >= 1:
                    emit_win_mm(2 * (p - 1))
                    emit_win_mm(2 * (p - 1) + 1)
                if g == NG - 1:
                    for idx in range(2):
                        tt = 2 * p + idx
                        pkv, psc = acc.pop(tt)
                        emit_epilogue(tt, pkv, psc)
            emit_win_mm(2 * (NPAIR - 1))
            emit_win_mm(2 * (NPAIR - 1) + 1)

    nc.compile()
    return nc


def _get_program():
    if "nc" not in _cache:
        _cache["nc"] = _build_program()
    return _cache["nc"]


def kernel(**inputs):
    hidden = np.ascontiguousarray(np.asarray(inputs["hidden_states"], np.float32))
    weight = np.asarray(inputs["weight"], np.float32)
    wscale = np.asarray(inputs["weight_scale"], np.float32)
    ape = np.asarray(inputs["ape"], np.float32)
    norm_weight = np.asarray(inputs["norm_weight"], np.float32)
    cos_sin = np.asarray(inputs["cos_sin_cache"], np.float32)
    state_cache = np.asarray(inputs["state_cache"], np.float32)
    kv_cache = np.asarray(inputs["kv_cache"], np.float32)
    positions = np.asarray(inputs["positions"])
    out_cache_loc = np.asarray(inputs["out_cache_loc"])
    kv_slot_mapping = np.asarray(inputs["kv_slot_mapping"])

    # --- sharding-structure checks (prefill layout: contiguous positions) ---
    for c in range(NCORES):
        p0 = int(positions[c * TPC])
        assert p0 % R == 0, "core token block must start on a window boundary"
        assert np.array_equal(
            positions[c * TPC : (c + 1) * TPC], np.arange(p0, p0 + TPC)
        ), "expected contiguous per-request positions"

    # --- host-side input relayout (no arithmetic on activations) ---
    wt = np.ascontiguousarray(weight.T)  # [H, 2D]
    wsc = np.broadcast_to(
        wscale.T.reshape(1, NKB * 8), (128, NKB * 8)
    ).copy()  # [p, kb*8+cb]
    ape_r = np.tile(ape, (128 // R, 1)).astype(np.float32)
    assert ape_r.shape == (128, D)
    ident = np.eye(128, dtype=np.float32)
    amat = np.zeros((128, 4 * 128), np.float32)
    tloc = np.arange(128)
    for i in range(4):
        amat[tloc, 128 * i + 32 * i + tloc // 4] = 1.0
    # per-window trig table with norm_weight folded in: [c*nw1 | s*nw2 | c*nw2 | s*nw1]
    cosv, sinv = cos_sin[:, : HD // 2], cos_sin[:, HD // 2 :]
    nw1, nw2 = norm_weight[: HD // 2], norm_weight[HD // 2 :]
    cs_table = np.concatenate(
        [cosv * nw1, sinv * nw2, cosv * nw2, sinv * nw1], axis=1
    ).astype(np.float32)  # [L//R, 512]

    in_maps = []
    wend_tokens = np.empty((NCORES, WPC), np.int64)
    for c in range(NCORES):
        tok0 = c * TPC
        wtok = tok0 + R * np.arange(WPC) + (R - 1)  # window-end tokens, window order
        wend_tokens[c] = wtok
        rows = positions[wtok] // R
        in_maps.append(
            {
                "h": hidden[tok0 : tok0 + TPC],
                "wt": wt,
                "wsc": wsc,
                "ape_r": ape_r,
                "cs2": np.ascontiguousarray(cs_table[rows]),
                "amat": amat,
                "ident": ident,
            }
        )

    from concourse.bass_utils import run_bass_kernel_spmd

    _cache["in_maps"] = in_maps
    nc = _get_program()
    res = run_bass_kernel_spmd(nc, in_maps, core_ids=list(range(NCORES)))
    results = res.results

    kv_score = np.concatenate([r["kv_score"] for r in results], axis=0)
    state = np.concatenate([r["state"] for r in results], axis=0)
    flat = np.concatenate([r["flat"] for r in results], axis=0)

    kv = np.ascontiguousarray(kv_score[:, :D])
    score = np.ascontiguousarray(kv_score[:, D:])
    sc = state_cache.copy()
    sc[out_cache_loc] = state
    kc = kv_cache.copy()
    kc[kv_slot_mapping[wend_tokens.reshape(-1)]] = flat
    return kv, score, sc, kc


# revision 15
# speedup vs baseline: 1.1615x; 1.1615x over previous
"""Trainium2 Bass kernel for nn_DeepseekV4Compressor (scatter_memory).

Computation (per the problem's nn.Module):
  w        = weight * block_scale_expand(weight_scale)     # fp8-style dequant
  kv_score = hidden_states @ w.T                           # [8192, 1024]
  kv, score = kv_score[:, :512], kv_score[:, 512:]
  state    = [kv + ape[pos%4] | score]  scattered into the paged state cache
  per compression window of R=4 tokens:
    gate = softmax(score_window, axis=window)  (per channel)
    comp = sum(gate * kvp)                                  # [2048, 512]
    x    = RoPE(RMSNorm(comp per 256-ch head), cos_sin[pos//4])
    kc[kv_slot_mapping[window_end]] = x

Sharding: tokens split contiguously across 8 cores (1024 tokens = half of one
request per core, so compression windows never straddle cores and all
scatter sources stay local).  The matmul runs in float32r (fp32 operands
truncated to ~fp22 in the PE) at full PE rate.  hidden is transposed on-chip
with PE transpose-mode matmuls; the weight is passed pre-transposed (pure
host-side relayout) and block-dequanted on the vector engine.  The window
softmax reduction (sum over 4 consecutive tokens living on adjacent
partitions) is computed as PE matmuls against a constant block-ones matrix.
The paged-cache scatters are applied during unsharding with the integer index
tensors (each row is produced exactly once, so scatter order is irrelevant).
"""

import numpy as np

NCORES = 8
T, H = 8192, 4096
D = 512            # coff * head_dim
D2 = 2 * D         # kv | score
HD = 256           # head dim
R = 4              # compression ratio
TPC = T // NCORES  # tokens per core = 1024
WPC = TPC // R     # windows per core = 256
NKB = H // 128     # 32 k-tiles of 128
NTT = TPC // 128   # 8 token tiles per core
NWT = WPC // 128   # 2 window tiles per core
NG = NKB // 4      # 8 transpose groups (4 k-tiles each) per token tile
EPS = 1e-6

_cache = {}


def _build_program():
    import concourse.bass as bass
    import concourse.tile as tile
    from concourse import bacc, mybir

    f32 = mybir.dt.float32
    f32r = mybir.dt.float32r
    AF = mybir.ActivationFunctionType

    nc = bacc.Bacc(
        "TRN2", target_bir_lowering=False, debug=False, num_devices=NCORES
    )

    h = nc.dram_tensor("h", [TPC, H], f32, kind="ExternalInput").ap()
    wt = nc.dram_tensor("wt", [H, D2], f32, kind="ExternalInput").ap()
    wsc = nc.dram_tensor("wsc", [128, NKB * 8], f32, kind="ExternalInput").ap()
    ape_r = nc.dram_tensor("ape_r", [128, D], f32, kind="ExternalInput").ap()
    cs2 = nc.dram_tensor("cs2", [WPC, 4 * 128], f32, kind="ExternalInput").ap()
    amat = nc.dram_tensor("amat", [128, 4 * 128], f32, kind="ExternalInput").ap()
    ident = nc.dram_tensor("ident", [128, 128], f32, kind="ExternalInput").ap()

    kv_score = nc.dram_tensor("kv_score", [TPC, D2], f32, kind="ExternalOutput").ap()
    state = nc.dram_tensor("state", [TPC, D2], f32, kind="ExternalOutput").ap()
    flat = nc.dram_tensor("flat", [WPC, D], f32, kind="ExternalOutput").ap()

    with tile.TileContext(nc) as tc:
        with (
            tc.tile_pool(name="consts", bufs=1) as consts,
            tc.tile_pool(name="wq", bufs=5) as wqp,
            tc.tile_pool(name="part", bufs=1) as partp,
            tc.tile_pool(name="hn", bufs=8) as hpool,
            tc.tile_pool(name="wst", bufs=3) as wstp,
            tc.tile_pool(name="ht", bufs=4) as htp,
            tc.tile_pool(name="ep", bufs=4) as epool,
            tc.tile_pool(name="pp", bufs=4) as ppool,
            tc.tile_pool(name="sp", bufs=3) as spool,
            tc.tile_pool(name="wep", bufs=1) as wepool,
            tc.tile_pool(name="tmp", bufs=1) as tmpp,
            tc.tile_pool(name="tp", bufs=2, space="PSUM") as tpp,
            tc.tile_pool(name="acc", bufs=4, space="PSUM") as accp,
            tc.tile_pool(name="win", bufs=2, space="PSUM") as winp,
        ):
            # ---- constants ----
            wsc_sb = consts.tile([128, NKB * 8], f32)
            nc.sync.dma_start(wsc_sb[:], wsc[:])
            ape_sb = consts.tile([128, D], f32)
            nc.sync.dma_start(ape_sb[:], ape_r[:])
            cs_sb = consts.tile([128, NWT, 4 * 128], f32)
            nc.sync.dma_start(cs_sb[:], cs2.rearrange("(a p) d -> p a d", p=128))
            am_r = consts.tile([128, 4 * 128], f32r)
            am_st = wstp.tile([128, 4 * 128], f32, tag="wst", name="am_st")
            nc.sync.dma_start(am_st[:], amat[:])
            nc.vector.tensor_copy(am_r[:], am_st[:])
            id_sb = consts.tile([128, 128], f32)
            nc.sync.dma_start(id_sb[:], ident[:])
            eps_sb = consts.tile([128, 1], f32)
            nc.vector.memset(eps_sb[:], EPS)

            # ---- weight chunks: 4 k-tiles per chunk, dequanted to f32r ----
            # wq[g][p, j*D2 + c] = w_deq[c, (4g+j)*128 + p]
            wq_tiles = {}

            def load_wchunk(g):
                t = wqp.tile([128, 4 * D2], f32r, tag="wq", name=f"wq{g}")
                for j in range(4):
                    kb = 4 * g + j
                    wstage = wstp.tile([128, D2], f32, tag="wst", name=f"wst{kb}")
                    nc.sync.dma_start(wstage[:], wt[kb * 128 : (kb + 1) * 128, :])
                    seg_out = t[:, j * D2 : (j + 1) * D2].rearrange(
                        "p (cb cc) -> p cb cc", cb=8
                    )
                    scl = (
                        wsc_sb[:, kb * 8 : (kb + 1) * 8]
                        .unsqueeze(2)
                        .broadcast_to([128, 8, 128])
                    )
                    nc.vector.tensor_mul(
                        seg_out,
                        wstage[:].rearrange("p (cb cc) -> p cb cc", cb=8),
                        scl,
                    )
                wq_tiles[g] = t

            def w_ap(g, j, chunk):
                t = wq_tiles[g]
                return t[:, j * D2 + chunk * D : j * D2 + (chunk + 1) * D]

            # ---- main pipeline: K-outermost super-chunks ----
            # Schedule: for super-chunk sc (8 k-tiles): for token-tile pair p:
            # transposes + matmuls of that k-range for both tiles.  Each
            # weight chunk is consumed by all pairs inside its super-chunk
            # window, so the weight stream needs only ~115 GB/s and the PE
            # never starves.  Between super-chunks the psum accumulators are
            # flushed into SBUF partials.
            SC_GROUPS = [[0, 1], [2, 3], [4, 5], [6, 7]]
            NPAIR = NTT // 2
            units = [
                (sc, p, gi)
                for sc in range(len(SC_GROUPS))
                for p in range(NPAIR)
                for gi in range(len(SC_GROUPS[sc]))
            ]
            hn_tiles = {}

            def load_hn(u):
                sc, p, gi = units[u]
                g = SC_GROUPS[sc][gi]
                for idx in range(2):
                    tt = 2 * p + idx
                    t = hpool.tile([128, 512], f32, tag="hn", name=f"hn{tt}_{g}")
                    eng = nc.sync if idx == 0 else nc.scalar
                    eng.dma_start(
                        t[:],
                        h[tt * 128 : (tt + 1) * 128, g * 512 : (g + 1) * 512],
                    )
                    hn_tiles[(tt, g)] = t

            ht_tiles = {}

            def emit_transpose_group(u, idx):
                sc, p, gi = units[u]
                g = SC_GROUPS[sc][gi]
                tt = 2 * p + idx
                hn = hn_tiles.pop((tt, g))
                pst = tpp.tile([128, 512], f32, tag="tp", name=f"pst{tt}_{g}")
                for j in range(4):
                    nc.tensor.matmul(
                        pst[:, j * 128 : (j + 1) * 128],
                        hn[:, j * 128 : (j + 1) * 128],
                        id_sb[:],
                        is_transpose=True,
                        skip_group_check=True,
                    )
                ht = htp.tile([128, 512], f32r, tag="ht", name=f"ht{tt}_{g}")
                nc.any.tensor_copy(out=ht[:], in_=pst[:])
                ht_tiles[(u, idx)] = ht

            def emit_mm(u, idx):
                sc, p, gi = units[u]
                g = SC_GROUPS[sc][gi]
                tt = 2 * p + idx
                pkv, psc = acc[tt]
                ht = ht_tiles.pop((u, idx))
                kb0 = SC_GROUPS[sc][0] * 4
                kb1 = SC_GROUPS[sc][-1] * 4 + 3
                for j in range(4):
                    kb = g * 4 + j
                    lhs = ht[:, j * 128 : (j + 1) * 128]
                    nc.tensor.matmul(
                        pkv, lhs, w_ap(g, j, 0), start=(kb == kb0), stop=(kb == kb1)
                    )
                    nc.tensor.matmul(
                        psc, lhs, w_ap(g, j, 1), start=(kb == kb0), stop=(kb == kb1)
                    )

            part_tiles = {}

            def emit_flush(sc, tt, pkv, psc):
                if sc == 0:
                    pt = partp.tile(
                        [128, D2], f32, tag=f"part{tt}", name=f"part{tt}"
                    )
                    part_tiles[tt] = pt
                    nc.any.tensor_copy(out=pt[:, 0:D], in_=pkv[:])
                    nc.any.tensor_copy(out=pt[:, D : 2 * D], in_=psc[:])
                else:
                    pt = part_tiles[tt]
                    nc.any.tensor_add(pt[:, 0:D], pt[:, 0:D], pkv[:])
                    nc.any.tensor_add(pt[:, D : 2 * D], pt[:, D : 2 * D], psc[:])

            ep_tiles = {}

            def emit_epilogue(tt, pkv, psc):
                pt = part_tiles.pop(tt)
                kvsc = spool.tile([128, 3 * D], f32, tag="sp")
                nc.vector.tensor_add(kvsc[:, 0:D], pkv[:], pt[:, 0:D])
                nc.any.tensor_add(kvsc[:, D : 2 * D], psc[:], pt[:, D : 2 * D])
                nc.vector.tensor_add(
                    kvsc[:, 2 * D : 3 * D], kvsc[:, 0:D], ape_sb[:]
                )
                rows = slice(tt * 128, (tt + 1) * 128)
                nc.sync.dma_start(kv_score[rows, :], kvsc[:, 0 : 2 * D])
                nc.sync.dma_start(state[rows, 0:D], kvsc[:, 2 * D : 3 * D])
                nc.sync.dma_start(state[rows, D : 2 * D], kvsc[:, D : 2 * D])
                E = epool.tile([128, D], f32r, tag="ep")
                nc.scalar.activation(E[:], kvsc[:, D : 2 * D], AF.Exp)
                P = ppool.tile([128, D], f32r, tag="pp")
                nc.vector.tensor_mul(
                    P[:], E[:].bitcast(f32), kvsc[:, 2 * D : 3 * D]
                )
                ep_tiles[tt] = (E, P)

            win_ps = {}

            def emit_win_mm(src_tt):
                wi, i = divmod(src_tt, 4)
                if i == 0:
                    win_ps[wi] = (
                        winp.tile([128, D], f32, tag="win", name=f"psN{wi}"),
                        winp.tile([128, D], f32, tag="win", name=f"psD{wi}"),
                    )
                psN, psD = win_ps[wi]
                E, P = ep_tiles.pop(src_tt)
                a_i = am_r[:, i * 128 : (i + 1) * 128]
                nc.tensor.matmul(psN, a_i, P[:], start=(i == 0), stop=(i == 3))
                nc.tensor.matmul(psD, a_i, E[:], start=(i == 0), stop=(i == 3))
                if i == 3:
                    emit_win_epilogue(wi)

            def emit_win_epilogue(wi):
                psN, psD = win_ps.pop(wi)
                rec = wepool.tile([128, D], f32, tag="rec")
                nc.vector.reciprocal(rec[:], psD[:])
                comp = wepool.tile([128, D], f32, tag="comp")
                nc.vector.tensor_mul(comp[:], psN[:], rec[:])
                sq = wepool.tile([128, HD], f32, tag="sq")
                ms = wepool.tile([128, 2], f32, tag="ms")
                nc.scalar.activation(
                    sq[:], comp[:, 0:HD], AF.Square, accum_out=ms[:, 0:1]
                )
                nc.scalar.activation(
                    sq[:], comp[:, HD : 2 * HD], AF.Square, accum_out=ms[:, 1:2]
                )
                rms = wepool.tile([128, 2], f32, tag="rms")
                nc.scalar.activation(
                    rms[:], ms[:], AF.Sqrt, scale=1.0 / HD, bias=eps_sb[:]
                )
                inv = wepool.tile([128, 2], f32, tag="inv")
                nc.vector.reciprocal(inv[:], rms[:])
                fl = wepool.tile([128, D], f32, tag="fl")
                for hh in range(2):
                    css = wepool.tile([128, 4 * 128], f32, tag="css")
                    nc.vector.tensor_scalar_mul(
                        css[:], cs_sb[:, wi], inv[:, hh : hh + 1]
                    )
                    x1 = comp[:, hh * HD : hh * HD + 128]
                    x2 = comp[:, hh * HD + 128 : (hh + 1) * HD]
                    t1 = tmpp.tile([128, 128], f32, tag="t1")
                    t2 = tmpp.tile([128, 128], f32, tag="t2")
                    nc.vector.tensor_mul(t1[:], x1, css[:, 0:128])
                    nc.vector.tensor_mul(t2[:], x2, css[:, 128:256])
                    nc.vector.tensor_sub(
                        fl[:, hh * HD : hh * HD + 128], t1[:], t2[:]
                    )
                    t3 = tmpp.tile([128, 128], f32, tag="t3")
                    t4 = tmpp.tile([128, 128], f32, tag="t4")
                    nc.vector.tensor_mul(t3[:], x2, css[:, 256:384])
                    nc.vector.tensor_mul(t4[:], x1, css[:, 384:512])
                    nc.vector.tensor_add(
                        fl[:, hh * HD + 128 : (hh + 1) * HD], t3[:], t4[:]
                    )
                nc.scalar.dma_start(flat[wi * 128 : (wi + 1) * 128, :], fl[:])

            # ---- emission ----
            total = len(units)
            acc = {}
            for g in range(3):
                load_wchunk(g)
            next_chunk = 3
            load_hn(0)
            load_hn(1)
            emit_transpose_group(0, 0)
            emit_transpose_group(0, 1)
            for u in range(total):
                sc, p, gi = units[u]
                last_gi = len(SC_GROUPS[sc]) - 1
                if gi == 0:
                    for idx in range(2):
                        tt = 2 * p + idx
                        acc[tt] = (
                            accp.tile(
                                [128, D], f32, tag="acc", name=f"pkv{tt}_{sc}"
                            ),
                            accp.tile(
                                [128, D], f32, tag="acc", name=f"psc{tt}_{sc}"
                            ),
                        )
                if u % 4 == 3 and next_chunk < NKB // 4:
                    load_wchunk(next_chunk)
                    next_chunk += 1
                if u + 2 < total:
                    load_hn(u + 2)
                if u + 1 < total:
                    emit_transpose_group(u + 1, 0)
                emit_mm(u, 0)
                if u + 1 < total:
                    emit_transpose_group(u + 1, 1)
                emit_mm(u, 1)
                if gi == last_gi:
                    for idx in range(2):
                        tt = 2 * p + idx
                        pkv, psc = acc.pop(tt)
                        if sc < len(SC_GROUPS) - 1:
                            emit_flush(sc, tt, pkv, psc)
                        else:
                            emit_epilogue(tt, pkv, psc)
                    if sc == len(SC_GROUPS) - 1 and p >= 1:
                        emit_win_mm(2 * (p - 1))
                        emit_win_mm(2 * (p - 1) + 1)
            emit_win_mm(2 * (NPAIR - 1))
            emit_win_mm(2 * (NPAIR - 1) + 1)

    nc.compile()
    return nc


def _get_program():
    if "nc" not in _cache:
        _cache["nc"] = _build_program()
    return _cache["nc"]


def kernel(**inputs):
    hidden = np.ascontiguousarray(np.asarray(inputs["hidden_states"], np.float32))
    weight = np.asarray(inputs["weight"], np.float32)
    wscale = np.asarray(inputs["weight_scale"], np.float32)
    ape = np.asarray(inputs["ape"], np.float32)
    norm_weight = np.asarray(inputs["norm_weight"], np.float32)
    cos_sin = np.asarray(inputs["cos_sin_cache"], np.float32)
    state_cache = np.asarray(inputs["state_cache"], np.float32)
    kv_cache = np.asarray(inputs["kv_cache"], np.float32)
    positions = np.asarray(inputs["positions"])
    out_cache_loc = np.asarray(inputs["out_cache_loc"])
    kv_slot_mapping = np.asarray(inputs["kv_slot_mapping"])

    # --- sharding-structure checks (prefill layout: contiguous positions) ---
    for c in range(NCORES):
        p0 = int(positions[c * TPC])
        assert p0 % R == 0, "core token block must start on a window boundary"
        assert np.array_equal(
            positions[c * TPC : (c + 1) * TPC], np.arange(p0, p0 + TPC)
        ), "expected contiguous per-request positions"

    # --- host-side input relayout (no arithmetic on activations) ---
    wt = np.ascontiguousarray(weight.T)  # [H, 2D]
    wsc = np.broadcast_to(
        wscale.T.reshape(1, NKB * 8), (128, NKB * 8)
    ).copy()  # [p, kb*8+cb]
    ape_r = np.tile(ape, (128 // R, 1)).astype(np.float32)
    assert ape_r.shape == (128, D)
    ident = np.eye(128, dtype=np.float32)
    amat = np.zeros((128, 4 * 128), np.float32)
    tloc = np.arange(128)
    for i in range(4):
        amat[tloc, 128 * i + 32 * i + tloc // 4] = 1.0
    # per-window trig table with norm_weight folded in: [c*nw1 | s*nw2 | c*nw2 | s*nw1]
    cosv, sinv = cos_sin[:, : HD // 2], cos_sin[:, HD // 2 :]
    nw1, nw2 = norm_weight[: HD // 2], norm_weight[HD // 2 :]
    cs_table = np.concatenate(
        [cosv * nw1, sinv * nw2, cosv * nw2, sinv * nw1], axis=1
    ).astype(np.float32)  # [L//R, 512]

    in_maps = []
    wend_tokens = np.empty((NCORES, WPC), np.int64)
    for c in range(NCORES):
        tok0 = c * TPC
        wtok = tok0 + R * np.arange(WPC) + (R - 1)  # window-end tokens, window order
        wend_tokens[c] = wtok
        rows = positions[wtok] // R
        in_maps.append(
            {
                "h": hidden[tok0 : tok0 + TPC],
                "wt": wt,
                "wsc": wsc,
                "ape_r": ape_r,
                "cs2": np.ascontiguousarray(cs_table[rows]),
                "amat": amat,
                "ident": ident,
            }
        )

    from concourse.bass_utils import run_bass_kernel_spmd

    _cache["in_maps"] = in_maps
    nc = _get_program()
    res = run_bass_kernel_spmd(nc, in_maps, core_ids=list(range(NCORES)))
    results = res.results

    kv_score = np.concatenate([r["kv_score"] for r in results], axis=0)
    state = np.concatenate([r["state"] for r in results], axis=0)
    flat = np.concatenate([r["flat"] for r in results], axis=0)

    kv = np.ascontiguousarray(kv_score[:, :D])
    score = np.ascontiguousarray(kv_score[:, D:])
    sc = state_cache.copy()
    sc[out_cache_loc] = state
    kc = kv_cache.copy()
    kc[kv_slot_mapping[wend_tokens.reshape(-1)]] = flat
    return kv, score, sc, kc


# revision 18
# speedup vs baseline: 1.2687x; 1.0923x over previous
"""Trainium2 Bass kernel for nn_DeepseekV4Compressor (scatter_memory).

Computation (per the problem's nn.Module):
  w        = weight * block_scale_expand(weight_scale)     # fp8-style dequant
  kv_score = hidden_states @ w.T                           # [8192, 1024]
  kv, score = kv_score[:, :512], kv_score[:, 512:]
  state    = [kv + ape[pos%4] | score]  scattered into the paged state cache
  per compression window of R=4 tokens:
    gate = softmax(score_window, axis=window)  (per channel)
    comp = sum(gate * kvp)                                  # [2048, 512]
    x    = RoPE(RMSNorm(comp per 256-ch head), cos_sin[pos//4])
    kc[kv_slot_mapping[window_end]] = x

Sharding: tokens split contiguously across 8 cores (1024 tokens = half of one
request per core, so compression windows never straddle cores and all
scatter sources stay local).  The matmul runs in float32r (fp32 operands
truncated to ~fp22 in the PE) at full PE rate.  hidden is transposed on-chip
with PE transpose-mode matmuls; the weight is passed pre-transposed (pure
host-side relayout) and block-dequanted on the vector engine.  The window
softmax reduction (sum over 4 consecutive tokens living on adjacent
partitions) is computed as PE matmuls against a constant block-ones matrix.
The paged-cache scatters are applied during unsharding with the integer index
tensors (each row is produced exactly once, so scatter order is irrelevant).
"""

import numpy as np

NCORES = 8
T, H = 8192, 4096
D = 512            # coff * head_dim
D2 = 2 * D         # kv | score
HD = 256           # head dim
R = 4              # compression ratio
TPC = T // NCORES  # tokens per core = 1024
WPC = TPC // R     # windows per core = 256
NKB = H // 128     # 32 k-tiles of 128
NTT = TPC // 128   # 8 token tiles per core
NWT = WPC // 128   # 2 window tiles per core
NG = NKB // 4      # 8 transpose groups (4 k-tiles each) per token tile
EPS = 1e-6

_cache = {}


def _build_program():
    import concourse.bass as bass
    import concourse.tile as tile
    from concourse import bacc, mybir

    f32 = mybir.dt.float32
    f32r = mybir.dt.float32r
    AF = mybir.ActivationFunctionType

    nc = bacc.Bacc(
        "TRN2", target_bir_lowering=False, debug=False, num_devices=NCORES
    )

    h = nc.dram_tensor("h", [TPC, H], f32, kind="ExternalInput").ap()
    wt = nc.dram_tensor("wt", [H, D2], f32, kind="ExternalInput").ap()
    wsc = nc.dram_tensor("wsc", [128, NKB * 8], f32, kind="ExternalInput").ap()
    ape_r = nc.dram_tensor("ape_r", [128, D], f32, kind="ExternalInput").ap()
    cs2 = nc.dram_tensor("cs2", [WPC, 4 * 128], f32, kind="ExternalInput").ap()
    amat = nc.dram_tensor("amat", [128, 4 * 128], f32, kind="ExternalInput").ap()
    ident = nc.dram_tensor("ident", [128, 128], f32, kind="ExternalInput").ap()

    kv_score = nc.dram_tensor("kv_score", [TPC, D2], f32, kind="ExternalOutput").ap()
    state = nc.dram_tensor("state", [TPC, D2], f32, kind="ExternalOutput").ap()
    flat = nc.dram_tensor("flat", [WPC, D], f32, kind="ExternalOutput").ap()

    with tile.TileContext(nc) as tc:
        with (
            tc.tile_pool(name="consts", bufs=1) as consts,
            tc.tile_pool(name="wq", bufs=4) as wqp,
            tc.tile_pool(name="part", bufs=1) as partp,
            tc.tile_pool(name="hn", bufs=8) as hpool,
            tc.tile_pool(name="wst", bufs=6) as wstp,
            tc.tile_pool(name="ht", bufs=4) as htp,
            tc.tile_pool(name="ep", bufs=4) as epool,
            tc.tile_pool(name="pp", bufs=4) as ppool,
            tc.tile_pool(name="sp", bufs=2) as spool,
            tc.tile_pool(name="wep", bufs=1) as wepool,
            tc.tile_pool(name="tmp", bufs=1) as tmpp,
            tc.tile_pool(name="tp", bufs=2, space="PSUM") as tpp,
            tc.tile_pool(name="acc", bufs=4, space="PSUM") as accp,
            tc.tile_pool(name="win", bufs=2, space="PSUM") as winp,
        ):
            # ---- constants ----
            wsc_sb = consts.tile([128, NKB * 8], f32)
            nc.sync.dma_start(wsc_sb[:], wsc[:])
            ape_sb = consts.tile([128, D], f32)
            nc.sync.dma_start(ape_sb[:], ape_r[:])
            cs_sb = consts.tile([128, NWT, 4 * 128], f32)
            nc.sync.dma_start(cs_sb[:], cs2.rearrange("(a p) d -> p a d", p=128))
            am_r = consts.tile([128, 4 * 128], f32r)
            am_st = wstp.tile([128, 4 * 128], f32, tag="wst", name="am_st")
            nc.sync.dma_start(am_st[:], amat[:])
            nc.vector.tensor_copy(am_r[:], am_st[:])
            id_sb = consts.tile([128, 128], f32)
            nc.sync.dma_start(id_sb[:], ident[:])
            eps_sb = consts.tile([128, 1], f32)
            nc.vector.memset(eps_sb[:], EPS)

            # ---- weight chunks: 4 k-tiles per chunk, dequanted to f32r ----
            # wq[g][p, j*D2 + c] = w_deq[c, (4g+j)*128 + p]
            wq_tiles = {}

            def load_wchunk(g):
                t = wqp.tile([128, 4 * D2], f32r, tag="wq", name=f"wq{g}")
                for j in range(4):
                    kb = 4 * g + j
                    wstage = wstp.tile([128, D2], f32, tag="wst", name=f"wst{kb}")
                    nc.sync.dma_start(wstage[:], wt[kb * 128 : (kb + 1) * 128, :])
                    seg_out = t[:, j * D2 : (j + 1) * D2].rearrange(
                        "p (cb cc) -> p cb cc", cb=8
                    )
                    scl = (
                        wsc_sb[:, kb * 8 : (kb + 1) * 8]
                        .unsqueeze(2)
                        .broadcast_to([128, 8, 128])
                    )
                    nc.vector.tensor_mul(
                        seg_out,
                        wstage[:].rearrange("p (cb cc) -> p cb cc", cb=8),
                        scl,
                    )
                wq_tiles[g] = t

            def w_ap(g, j, chunk):
                t = wq_tiles[g]
                return t[:, j * D2 + chunk * D : j * D2 + (chunk + 1) * D]

            # ---- main pipeline: K-outermost super-chunks ----
            # Schedule: for super-chunk sc (8 k-tiles): for token-tile pair p:
            # transposes + matmuls of that k-range for both tiles.  Each
            # weight chunk is consumed by all pairs inside its super-chunk
            # window, so the weight stream needs only ~115 GB/s and the PE
            # never starves.  Between super-chunks the psum accumulators are
            # flushed into SBUF partials.
            SC_GROUPS = [[0, 1], [2, 3], [4, 5], [6, 7]]
            NPAIR = NTT // 2
            units = [
                (sc, p, gi)
                for sc in range(len(SC_GROUPS))
                for p in range(NPAIR)
                for gi in range(len(SC_GROUPS[sc]))
            ]
            hn_tiles = {}

            def load_hn(u):
                sc, p, gi = units[u]
                g = SC_GROUPS[sc][gi]
                for idx in range(2):
                    tt = 2 * p + idx
                    t = hpool.tile([128, 512], f32, tag="hn", name=f"hn{tt}_{g}")
                    eng = nc.sync if idx == 0 else nc.scalar
                    eng.dma_start(
                        t[:],
                        h[tt * 128 : (tt + 1) * 128, g * 512 : (g + 1) * 512],
                    )
                    hn_tiles[(tt, g)] = t

            ht_tiles = {}

            def emit_transpose_group(u, idx):
                sc, p, gi = units[u]
                g = SC_GROUPS[sc][gi]
                tt = 2 * p + idx
                hn = hn_tiles.pop((tt, g))
                pst = tpp.tile([128, 512], f32, tag="tp", name=f"pst{tt}_{g}")
                for j in range(4):
                    nc.tensor.matmul(
                        pst[:, j * 128 : (j + 1) * 128],
                        hn[:, j * 128 : (j + 1) * 128],
                        id_sb[:],
                        is_transpose=True,
                        skip_group_check=True,
                    )
                ht = htp.tile([128, 512], f32r, tag="ht", name=f"ht{tt}_{g}")
                nc.any.tensor_copy(out=ht[:], in_=pst[:])
                ht_tiles[(u, idx)] = ht

            def emit_mm(u, idx):
                sc, p, gi = units[u]
                g = SC_GROUPS[sc][gi]
                tt = 2 * p + idx
                pkv, psc = acc[tt]
                ht = ht_tiles.pop((u, idx))
                kb0 = SC_GROUPS[sc][0] * 4
                kb1 = SC_GROUPS[sc][-1] * 4 + 3
                for j in range(4):
                    kb = g * 4 + j
                    lhs = ht[:, j * 128 : (j + 1) * 128]
                    nc.tensor.matmul(
                        pkv, lhs, w_ap(g, j, 0), start=(kb == kb0), stop=(kb == kb1)
                    )
                    nc.tensor.matmul(
                        psc, lhs, w_ap(g, j, 1), start=(kb == kb0), stop=(kb == kb1)
                    )

            part_tiles = {}

            def emit_flush(sc, tt, pkv, psc):
                if sc == 0:
                    pt = partp.tile(
                        [128, D2], f32, tag=f"part{tt}", name=f"part{tt}"
                    )
                    part_tiles[tt] = pt
                    nc.any.tensor_copy(out=pt[:, 0:D], in_=pkv[:])
                    nc.any.tensor_copy(out=pt[:, D : 2 * D], in_=psc[:])
                else:
                    pt = part_tiles[tt]
                    nc.any.tensor_add(pt[:, 0:D], pt[:, 0:D], pkv[:])
                    nc.any.tensor_add(pt[:, D : 2 * D], pt[:, D : 2 * D], psc[:])

            ep_tiles = {}

            def emit_epilogue(tt, pkv, psc):
                pt = part_tiles.pop(tt)
                kvsc = spool.tile([128, 3 * D], f32, tag="sp")
                nc.vector.tensor_add(kvsc[:, 0:D], pkv[:], pt[:, 0:D])
                nc.any.tensor_add(kvsc[:, D : 2 * D], psc[:], pt[:, D : 2 * D])
                nc.vector.tensor_add(
                    kvsc[:, 2 * D : 3 * D], kvsc[:, 0:D], ape_sb[:]
                )
                rows = slice(tt * 128, (tt + 1) * 128)
                nc.sync.dma_start(kv_score[rows, :], kvsc[:, 0 : 2 * D])
                nc.sync.dma_start(state[rows, 0:D], kvsc[:, 2 * D : 3 * D])
                nc.sync.dma_start(state[rows, D : 2 * D], kvsc[:, D : 2 * D])
                E = epool.tile([128, D], f32r, tag="ep")
                nc.scalar.activation(E[:], kvsc[:, D : 2 * D], AF.Exp)
                P = ppool.tile([128, D], f32r, tag="pp")
                nc.vector.tensor_mul(
                    P[:], E[:].bitcast(f32), kvsc[:, 2 * D : 3 * D]
                )
                ep_tiles[tt] = (E, P)

            win_ps = {}

            def emit_win_mm(src_tt):
                wi, i = divmod(src_tt, 4)
                if i == 0:
                    win_ps[wi] = (
                        winp.tile([128, D], f32, tag="win", name=f"psN{wi}"),
                        winp.tile([128, D], f32, tag="win", name=f"psD{wi}"),
                    )
                psN, psD = win_ps[wi]
                E, P = ep_tiles.pop(src_tt)
                a_i = am_r[:, i * 128 : (i + 1) * 128]
                nc.tensor.matmul(psN, a_i, P[:], start=(i == 0), stop=(i == 3))
                nc.tensor.matmul(psD, a_i, E[:], start=(i == 0), stop=(i == 3))
                if i == 3:
                    emit_win_epilogue(wi)

            def emit_win_epilogue(wi):
                psN, psD = win_ps.pop(wi)
                rec = wepool.tile([128, D], f32, tag="rec")
                nc.vector.reciprocal(rec[:], psD[:])
                comp = wepool.tile([128, D], f32, tag="comp")
                nc.vector.tensor_mul(comp[:], psN[:], rec[:])
                sq = wepool.tile([128, HD], f32, tag="sq")
                ms = wepool.tile([128, 2], f32, tag="ms")
                nc.scalar.activation(
                    sq[:], comp[:, 0:HD], AF.Square, accum_out=ms[:, 0:1]
                )
                nc.scalar.activation(
                    sq[:], comp[:, HD : 2 * HD], AF.Square, accum_out=ms[:, 1:2]
                )
                rms = wepool.tile([128, 2], f32, tag="rms")
                nc.scalar.activation(
                    rms[:], ms[:], AF.Sqrt, scale=1.0 / HD, bias=eps_sb[:]
                )
                inv = wepool.tile([128, 2], f32, tag="inv")
                nc.vector.reciprocal(inv[:], rms[:])
                fl = wepool.tile([128, D], f32, tag="fl")
                for hh in range(2):
                    css = wepool.tile([128, 4 * 128], f32, tag="css")
                    nc.vector.tensor_scalar_mul(
                        css[:], cs_sb[:, wi], inv[:, hh : hh + 1]
                    )
                    x1 = comp[:, hh * HD : hh * HD + 128]
                    x2 = comp[:, hh * HD + 128 : (hh + 1) * HD]
                    t1 = tmpp.tile([128, 128], f32, tag="t1")
                    t2 = tmpp.tile([128, 128], f32, tag="t2")
                    nc.vector.tensor_mul(t1[:], x1, css[:, 0:128])
                    nc.vector.tensor_mul(t2[:], x2, css[:, 128:256])
                    nc.vector.tensor_sub(
                        fl[:, hh * HD : hh * HD + 128], t1[:], t2[:]
                    )
                    t3 = tmpp.tile([128, 128], f32, tag="t3")
                    t4 = tmpp.tile([128, 128], f32, tag="t4")
                    nc.vector.tensor_mul(t3[:], x2, css[:, 256:384])
                    nc.vector.tensor_mul(t4[:], x1, css[:, 384:512])
                    nc.vector.tensor_add(
                        fl[:, hh * HD + 128 : (hh + 1) * HD], t3[:], t4[:]
                    )
                nc.scalar.dma_start(flat[wi * 128 : (wi + 1) * 128, :], fl[:])

            # ---- emission ----
            total = len(units)
            acc = {}
            load_hn(0)
            load_hn(1)
            for g in range(3):
                load_wchunk(g)
            next_chunk = 3
            emit_transpose_group(0, 0)
            emit_transpose_group(0, 1)
            for u in range(total):
                sc, p, gi = units[u]
                last_gi = len(SC_GROUPS[sc]) - 1
                if gi == 0:
                    for idx in range(2):
                        tt = 2 * p + idx
                        acc[tt] = (
                            accp.tile(
                                [128, D], f32, tag="acc", name=f"pkv{tt}_{sc}"
                            ),
                            accp.tile(
                                [128, D], f32, tag="acc", name=f"psc{tt}_{sc}"
                            ),
                        )
                if u % 4 == 3 and next_chunk < NKB // 4:
                    load_wchunk(next_chunk)
                    next_chunk += 1
                if u + 2 < total:
                    load_hn(u + 2)
                if u + 1 < total:
                    emit_transpose_group(u + 1, 0)
                emit_mm(u, 0)
                if u + 1 < total:
                    emit_transpose_group(u + 1, 1)
                emit_mm(u, 1)
                if gi == last_gi:
                    for idx in range(2):
                        tt = 2 * p + idx
                        pkv, psc = acc.pop(tt)
                        if sc < len(SC_GROUPS) - 1:
                            emit_flush(sc, tt, pkv, psc)
                        else:
                            emit_epilogue(tt, pkv, psc)
                    if sc == len(SC_GROUPS) - 1 and p >= 1:
                        emit_win_mm(2 * (p - 1))
                        emit_win_mm(2 * (p - 1) + 1)
            emit_win_mm(2 * (NPAIR - 1))
            emit_win_mm(2 * (NPAIR - 1) + 1)

    nc.compile()
    return nc


def _get_program():
    if "nc" not in _cache:
        _cache["nc"] = _build_program()
    return _cache["nc"]


def kernel(**inputs):
    hidden = np.ascontiguousarray(np.asarray(inputs["hidden_states"], np.float32))
    weight = np.asarray(inputs["weight"], np.float32)
    wscale = np.asarray(inputs["weight_scale"], np.float32)
    ape = np.asarray(inputs["ape"], np.float32)
    norm_weight = np.asarray(inputs["norm_weight"], np.float32)
    cos_sin = np.asarray(inputs["cos_sin_cache"], np.float32)
    state_cache = np.asarray(inputs["state_cache"], np.float32)
    kv_cache = np.asarray(inputs["kv_cache"], np.float32)
    positions = np.asarray(inputs["positions"])
    out_cache_loc = np.asarray(inputs["out_cache_loc"])
    kv_slot_mapping = np.asarray(inputs["kv_slot_mapping"])

    # --- sharding-structure checks (prefill layout: contiguous positions) ---
    for c in range(NCORES):
        p0 = int(positions[c * TPC])
        assert p0 % R == 0, "core token block must start on a window boundary"
        assert np.array_equal(
            positions[c * TPC : (c + 1) * TPC], np.arange(p0, p0 + TPC)
        ), "expected contiguous per-request positions"

    # --- host-side input relayout (no arithmetic on activations) ---
    wt = np.ascontiguousarray(weight.T)  # [H, 2D]
    wsc = np.broadcast_to(
        wscale.T.reshape(1, NKB * 8), (128, NKB * 8)
    ).copy()  # [p, kb*8+cb]
    ape_r = np.tile(ape, (128 // R, 1)).astype(np.float32)
    assert ape_r.shape == (128, D)
    ident = np.eye(128, dtype=np.float32)
    amat = np.zeros((128, 4 * 128), np.float32)
    tloc = np.arange(128)
    for i in range(4):
        amat[tloc, 128 * i + 32 * i + tloc // 4] = 1.0
    # per-window trig table with norm_weight folded in: [c*nw1 | s*nw2 | c*nw2 | s*nw1]
    cosv, sinv = cos_sin[:, : HD // 2], cos_sin[:, HD // 2 :]
    nw1, nw2 = norm_weight[: HD // 2], norm_weight[HD // 2 :]
    cs_table = np.concatenate(
        [cosv * nw1, sinv * nw2, cosv * nw2, sinv * nw1], axis=1
    ).astype(np.float32)  # [L//R, 512]

    in_maps = []
    wend_tokens = np.empty((NCORES, WPC), np.int64)
    for c in range(NCORES):
        tok0 = c * TPC
        wtok = tok0 + R * np.arange(WPC) + (R - 1)  # window-end tokens, window order
        wend_tokens[c] = wtok
        rows = positions[wtok] // R
        in_maps.append(
            {
                "h": hidden[tok0 : tok0 + TPC],
                "wt": wt,
                "wsc": wsc,
                "ape_r": ape_r,
                "cs2": np.ascontiguousarray(cs_table[rows]),
                "amat": amat,
                "ident": ident,
            }
        )

    from concourse.bass_utils import run_bass_kernel_spmd

    _cache["in_maps"] = in_maps
    nc = _get_program()
    res = run_bass_kernel_spmd(nc, in_maps, core_ids=list(range(NCORES)))
    results = res.results

    kv_score = np.concatenate([r["kv_score"] for r in results], axis=0)
    state = np.concatenate([r["state"] for r in results], axis=0)
    flat = np.concatenate([r["flat"] for r in results], axis=0)

    kv = np.ascontiguousarray(kv_score[:, :D])
    score = np.ascontiguousarray(kv_score[:, D:])
    sc = state_cache.copy()
    sc[out_cache_loc] = state
    kc = kv_cache.copy()
    kc[kv_slot_mapping[wend_tokens.reshape(-1)]] = flat
    return kv, score, sc, kc
